# revision 50
# baseline (speedup 1.0000x reference)
"""CrossAttentionBlock Trainium2 kernel (v2).

Math (reference):
    q = Wq@xq + bq        [RC=16, N]     (per-voxel 1x1x1 conv == channel matmul)
    k = Wk@xkv + bk       [16, N]
    v = Wv@xkv + bv       [C=128, N]
    S = (q^T k) / 4       [N, N]
    P = softmax_rows(S)
    out = v @ P^T         [C, N]
    y = x_q + gamma*out

Kernel strategy (8 NeuronCores, sequence-parallel over the N=13824 query
tokens; each core owns NQ=1728 queries against full K/V):
  * The hard throughput floor is PSUM->SBUF evacuation bandwidth: only the
    Activation and DVE engines can read PSUM, and every exp'd score element
    must cross once (exp is fused into the evacuation op, so exp itself is
    free).  Everything else is arranged to keep that path minimal:
      - v is never materialized: out = (gamma*Wv) @ (xkv @ P^T) reassociated,
        so the big [C,N] v evacuation disappears; Z = xkv @ exp(S^T)
        accumulates in PSUM via the same per-pair matmuls and is evacuated
        once per chunk ([C,432] instead of [C,N]).
      - the k projection packs 3 column-groups of 16 output rows into one
        [128,512] PSUM tile (base partitions 0/32/64), so its evacuation
        runs with full 128-lane utilization; a few SBUF->SBUF DMAs remap to
        the DoubleRow layout afterwards.
  * Scores are built TRANSPOSED (S^T tiles [128 keys x 432 queries]), fp8 +
    DoubleRow everywhere (2 MACs/cell/cycle): k is host-scaled by 16 into
    fp8 weights, so exp applies a 1/16 input scale (ScalarE scale arg /
    folded into the Schraudolph constant on DVE).  No max subtraction
    (|S|<~2 by construction); normalization deferred: Z and a ones-row
    matmul (ones=0.25 folds the Z evac scale) accumulate per chunk, then
    reciprocal + 1->128 broadcast matmul + multiply + residual add.
  * exp alternates ScalarE (true exp, fp8 out) / VectorE (Schraudolph int8
    bit-trick) ~53/47 Bresenham-interleaved.  Inputs land fp8/bf16 (xkv in
    both [c,m] and [m,c] layouts), chunked DMAs so projections overlap the
    loads.  Attention contributes O(1e-4) of the output, so fp8 noise is
    invisible; the residual is bf16 (0.2% of tolerance).
"""

import contextlib

import numpy as np
import ml_dtypes

import concourse.bass as bass
import concourse.mybir as mybir
from concourse import bacc
from concourse.tile import TileContext
from concourse.bass_utils import run_bass_kernel_spmd

F32 = mybir.dt.float32
BF16 = mybir.dt.bfloat16
FP8 = mybir.dt.float8e4
I8 = mybir.dt.int8
AF = mybir.ActivationFunctionType
DR = mybir.MatmulPerfMode.DoubleRow
ALU = mybir.AluOpType

C = 128           # channels
RC = 16           # reduced (q/k) channels
D = H = W = 24
N = D * H * W     # 13824 tokens
NCORES = 8
NQ = N // NCORES  # 1728 queries per core
CHUNK = 432       # query chunk ([128, CHUNK] f32 fits half a PSUM slot)
NCHUNKS = NQ // CHUNK   # 4
MT = N // 128     # 108 key tiles of 128
PAIRS = MT // 2   # 54 key-tile pairs per chunk
LAGP = 6          # Z/rs matmuls trail exp by this many pairs

KTW = 512         # k-projection column width per matmul (one PSUM bank)
KGROUPS = 3       # k output-row groups per PSUM tile (base partitions 0/32/64)
KTILES = N // (KTW * KGROUPS)   # 9

LOG2E = 1.4426950408889634
EXP8_SCALE = 8.0 * LOG2E / 16.0   # e4m3 bit trick, folding the S'=16*S scale
EXP8_BIAS = 56.0 - 0.3            # 7*8 + Schraudolph offset
ACT_FRAC = 0.531                  # ScalarE share of exp ops (1025/(905+1025))


def _act_pattern(n):
    pat, acc = [], 0.0
    for _ in range(n):
        acc += ACT_FRAC
        if acc >= 1.0:
            acc -= 1.0
            pat.append(True)
        else:
            pat.append(False)
    return pat

_BUILD_CACHE: dict = {}


def build_nc(repeats: int = 1):
    """Build + compile the per-core Bass program (SPMD across 8 cores)."""
    key = repeats
    if key in _BUILD_CACHE:
        return _BUILD_CACHE[key]

    nc = bacc.Bacc("TRN2", target_bir_lowering=False, debug=False,
                   num_devices=NCORES)
    wbf = nc.dram_tensor("wbf", [C, 2 * RC + C], BF16, kind="ExternalInput").ap()
    bias2 = nc.dram_tensor("bias2", [C, 3], F32, kind="ExternalInput").ap()
    xkv_f8 = nc.dram_tensor("xkv_f8", [C, N], FP8, kind="ExternalInput").ap()
    xq_bf = nc.dram_tensor("xq_bf", [C, NQ], BF16, kind="ExternalInput").ap()
    xkvT = nc.dram_tensor("xkvT", [C, N], FP8, kind="ExternalInput").ap()
    y = nc.dram_tensor("y", [C, NQ], BF16, kind="ExternalOutput").ap()

    with TileContext(nc) as tc, contextlib.ExitStack() as ctx:
        cpool = ctx.enter_context(tc.tile_pool(name="consts", bufs=1))
        ppool = ctx.enter_context(tc.tile_pool(name="psum", bufs=1, space="PSUM"))
        spool = ctx.enter_context(tc.tile_pool(name="work", bufs=1))

        # ---- resident inputs (issue order == HWDGE order: critical first) --
        KT1 = KTW * KGROUPS          # one k-tile's worth of xkv columns
        wbf_sb = cpool.tile([C, 2 * RC + C], BF16)
        nc.sync.dma_start(wbf_sb[:], wbf[:])
        bias_sb = cpool.tile([C, 3], F32)
        nc.sync.dma_start(bias_sb[:], bias2[:])
        xq_sb = cpool.tile([C, NQ], BF16)
        nc.sync.dma_start(xq_sb[:, 0:CHUNK], xq_bf[:, 0:CHUNK])
        xkv_sb = cpool.tile([C, N], FP8)
        # small chunks first so the k projection starts early; the big tail
        # chunks are issued AFTER the first k remaps so the remap transfers
        # aren't stuck behind them on the (exclusive) DMA engines.
        nc.sync.dma_start(xkv_sb[:, 0:KT1], xkv_f8[:, 0:KT1])
        nc.sync.dma_start(xkv_sb[:, KT1:2 * KT1], xkv_f8[:, KT1:2 * KT1])
        nc.sync.dma_start(xq_sb[:, CHUNK:NQ], xq_bf[:, CHUNK:NQ])
        XH = (N - 2 * KT1) // 2
        nc.sync.dma_start(xkv_sb[:, 2 * KT1:2 * KT1 + XH],
                          xkv_f8[:, 2 * KT1:2 * KT1 + XH])
        nc.sync.dma_start(xkv_sb[:, 2 * KT1 + XH:N], xkv_f8[:, 2 * KT1 + XH:N])
        xkvT_sb = cpool.tile([C, N], FP8)

        wqT = wbf_sb[:, 0:RC]
        wkT = wbf_sb[:, RC:2 * RC]
        wvT = wbf_sb[:, 2 * RC:2 * RC + C]
        bk16 = bias_sb[:, 0:1]
        bq_lo = bias_sb[0:8, 1:2]   # bq[p]/4 on partition p
        bq_hi = bias_sb[0:8, 2:3]   # bq[8+p]/4 on partition p

        ones_db = cpool.tile([C, 32], FP8)
        nc.gpsimd.memset(ones_db[:], 0.0625)   # folds the Z-evac 1/16 scale
        ones_row = cpool.tile([1, C], BF16)  # lhsT for 1->128 broadcast matmul
        nc.gpsimd.memset(ones_row[:], 1.0)
        warm_mv = cpool.tile([1, 512], BF16)
        nc.gpsimd.memset(warm_mv[:], 0.0)

        # PE p-state warmup: keep PE busy early so projection matmuls run at
        # full clock once their inputs land.
        warm_ps = ppool.tile([C, 512], F32, tag="rs", bufs=1, name="warm_ps")
        for _ in range(6):
            nc.tensor.matmul(warm_ps[0:1, :], ones_row[:, 0:1], warm_mv[:],
                             start=True, stop=True)


        # manual rotating PSUM arena: 3 slots x [C, 1024] f32 (2 banks each)
        st_arena = ppool.tile([C, 3 * 1024], F32, tag="starena", name="st_arena")
        _slot = [0]

        def st_slot():
            i = _slot[0] % 3
            _slot[0] += 1
            return i, st_arena[:, 1024 * i:1024 * (i + 1)]

        # greedy engine balancing on projected finish time
        eng_t = {"act": 1283.0, "dve": 0.0}  # ACT starts with the table load

        def pick(cost_act, cost_dve):
            if eng_t["act"] + cost_act <= eng_t["dve"] + cost_dve:
                eng_t["act"] += cost_act
                return True
            eng_t["dve"] += cost_dve
            return False

        # exp emission: pairs in arena slots 0/1 wait for a partner so one op
        # covers two supertiles; slot-2 pairs (and fusion breaks) go single
        fuse_pend = []
        ex_tiles = {}

        def _emit_exp(view, width, cost_act, cost_dve):
            ex = spool.tile([C, width * CHUNK], FP8, tag=f"ex{width}", bufs=5,
                            name="ex")
            exv = ex.rearrange("p (q x) -> p q x", q=width)
            if pick(cost_act, cost_dve):
                nc.scalar.activation(exv, view, AF.Exp, scale=1.0 / 16.0)
            else:
                nc.vector.tensor_scalar(out=exv.bitcast(I8), in0=view,
                                        scalar1=EXP8_SCALE, scalar2=EXP8_BIAS,
                                        op0=ALU.mult, op1=ALU.add)
            return ex

        def flush_exp():
            if not fuse_pend:
                return
            gp0, i0 = fuse_pend.pop()
            view = st_arena[:, 1024 * i0:1024 * (i0 + 1)].rearrange(
                "p (q x) -> p q x", q=2)[:, :, 0:CHUNK]
            ex = _emit_exp(view, 2, 905.0, 1025.0)
            ex_tiles[gp0] = (ex, 2, 0)

        def pair_exp(gp, i):
            if fuse_pend and fuse_pend[0][1] + 1 == i:
                gp0, i0 = fuse_pend.pop()
                view = st_arena[:, 1024 * i0:1024 * i0 + 2048].rearrange(
                    "p (q x) -> p q x", q=4)[:, :, 0:CHUNK]
                ex = _emit_exp(view, 4, 1625.0, 1925.0)
                ex_tiles[gp0] = (ex, 4, 0)
                ex_tiles[gp] = (ex, 4, 2)
            else:
                fuse_pend.append((gp, i))
                if i == 2:
                    flush_exp()

        # ---- projections ---------------------------------------------------
        # k': [128, KTILES*KTW] fp8, partition 32g+r holds 16*k[r] for column
        # group g; evacuations run full-width, then SBUF->SBUF DMAs remap to
        # the DoubleRow layout (o=0 on HWDGE/SP, o=1 on SWDGE/Pool).  q': two
        # 8-row matmuls per chunk-group write the DoubleRow halves side by
        # side in PSUM; 8-lane evacs land straight in q_db layout (no remap).
        # Only k tiles 0-1 and q group 0 run before the attention loop; the
        # rest is interleaved into chunk 0's pair pipeline below.
        k_sb = cpool.tile([C, KTILES * KTW], FP8)
        q_db = cpool.tile([8, 2 * NQ], FP8)
        qdv = q_db.rearrange("p (o g m) -> p o g m", o=2, g=NCHUNKS)
        k_db = cpool.tile([8, 2 * N], FP8)
        kv = k_sb.rearrange("p (t m) -> p t m", t=KTILES)
        kdv = k_db.rearrange("p (o t g m) -> p o t g m", o=2, t=KTILES, g=KGROUPS)

        def q_proj(g):
            flush_exp()
            _, psq = st_slot()
            for o in range(2):
                nc.tensor.matmul(psq[0:8, 512 * o:512 * o + CHUNK],
                                 wqT[:, 8 * o:8 * o + 8],
                                 xq_sb[:, bass.ts(g, CHUNK)],
                                 start=True, stop=True)
            for o, b in ((0, bq_lo), (1, bq_hi)):
                src = psq[0:8, 512 * o:512 * o + CHUNK]
                if pick(545, 575):
                    nc.scalar.activation(qdv[:, o, g, :], src, AF.Identity,
                                         bias=b)
                else:
                    nc.vector.tensor_scalar(out=qdv[:, o, g, :], in0=src,
                                            scalar1=b, scalar2=None,
                                            op0=ALU.add)

        def k_tile(t):
            flush_exp()
            _, psk = st_slot()
            for g in range(KGROUPS):
                lo = (KGROUPS * t + g) * KTW
                nc.tensor.matmul(psk[32 * g:32 * g + RC, 0:KTW],
                                 wkT, xkv_sb[:, lo:lo + KTW],
                                 start=True, stop=True)
            dst = k_sb[:, bass.ts(t, KTW)]
            if pick(612, 658):
                nc.scalar.activation(dst, psk[:, 0:KTW], AF.Identity, bias=bk16)
            else:
                nc.vector.tensor_scalar(out=dst, in0=psk[:, 0:KTW],
                                        scalar1=bk16, scalar2=None, op0=ALU.add)

        def k_remap(t0, t1, pool_frac=False):
            # critical remaps all ride HWDGE (SP); late batches push the o=1
            # half through SWDGE (Pool) to keep HWDGE clear for xkvT loads
            tsl = slice(t0, t1)
            for g in range(KGROUPS):
                nc.sync.dma_start(kdv[:, 0, tsl, g, :],
                                  kv[32 * g:32 * g + 8, tsl, :])
                eng = nc.gpsimd if pool_frac else nc.sync
                eng.dma_start(kdv[:, 1, tsl, g, :],
                              kv[32 * g + 8:32 * g + 16, tsl, :])

        def xkvT_load(qq):
            sl = bass.ts(qq, N // 4)
            nc.sync.dma_start(xkvT_sb[:, sl], xkvT[:, sl])

        k_tile(0)
        q_proj(0)
        k_tile(1)
        k_remap(0, 2, True)
        xkvT_load(0)

        q3 = q_db.rearrange("p (o x) -> p o x", o=2)
        k3 = k_db.rearrange("p (o x) -> p o x", o=2)
        # remaining projection work, interleaved at chunk-0 pair slots
        extras = {
            0: [lambda: k_tile(2)],
            2: [lambda: k_tile(3)],
            4: [lambda: k_remap(2, 4, True)],
            5: [lambda: xkvT_load(1)],
            6: [lambda: k_tile(4)],
            8: [lambda: k_tile(5)],
            10: [lambda: k_tile(6)],
            12: [lambda: k_tile(7)],
            14: [lambda: k_tile(8)],
            15: [lambda: q_proj(1), lambda: k_remap(4, KTILES, True)],
            16: [lambda: xkvT_load(2)],
            17: [lambda: q_proj(2)],
            18: [lambda: xkvT_load(3)],
            19: [lambda: q_proj(3)],
        }

        # ---- attention main loop ------------------------------------------
        # The per-chunk epilogue is deferred into the NEXT chunk's pipeline.
        pend = {}

        def epi_a():
            eng_t["act"] += 545.0
            eng_t["dve"] += 575.0
            # evacuate Z (frees its bank for the next chunk) + reciprocal
            pend["z8"] = z8 = spool.tile([C, CHUNK], FP8, tag="z8", bufs=2,
                                         name="z8")
            nc.scalar.activation(z8[:], pend.pop("z")[:], AF.Copy,
                                 scale=1.0 / 16.0)
            recip = spool.tile([1, CHUNK], F32, tag="recip", bufs=2)
            nc.vector.reciprocal_approx_fast(out=recip[:], in_=pend.pop("rs")[:])
            pend["recip_bf"] = recip_bf = spool.tile([1, CHUNK], BF16,
                                                     tag="recipb", bufs=2,
                                                     name="recip_bf")
            nc.gpsimd.tensor_copy(recip_bf[:], recip[:])

        def epi_m():
            # borrow one arena slot: outu (cols 0:CHUNK) + bcp (512:512+CHUNK)
            eng_t["act"] += 545.0
            flush_exp()
            _, pet = st_slot()
            pend["pe_t"] = pet
            nc.tensor.matmul(pet[:, 0:CHUNK], wvT, pend.pop("z8")[:],
                             start=True, stop=True)
            nc.tensor.matmul(pet[:, 512:512 + CHUNK], ones_row[:],
                             pend.pop("recip_bf")[:], start=True, stop=True)
            pend["outu_s"] = outu_s = spool.tile([C, CHUNK], F32, tag="outu_s",
                                                 bufs=2, name="outu_s")
            nc.scalar.copy(outu_s[:], pet[:, 0:CHUNK])

        def epi_b():
            eng_t["dve"] += 575.0
            sl = pend.pop("sl")
            pet = pend.pop("pe_t")
            t1 = spool.tile([C, CHUNK], BF16, tag="t1", bufs=2)
            nc.vector.tensor_tensor(out=t1[:], in0=pet[:, 512:512 + CHUNK],
                                    in1=pend.pop("outu_s")[:], op=ALU.mult)
            res = spool.tile([C, CHUNK], BF16, tag="res", bufs=2)
            nc.gpsimd.tensor_add(res[:], t1[:], xq_sb[:, sl])
            nc.sync.dma_start(y[:, sl], res[:])

        def epi_final(ch):
            # tail-latency version: two column halves pipelined across
            # engines, one y DMA at the end
            z, rs = pend.pop("z"), pend.pop("rs")
            HW_ = CHUNK // 2
            resf = spool.tile([C, CHUNK], BF16, tag="resf", bufs=1, name="rsf")
            for h in range(2):
                lo = h * HW_
                z8h = spool.tile([C, HW_], FP8, tag="z8f", bufs=2, name="z8f")
                nc.scalar.activation(z8h[:], z[:, lo:lo + HW_], AF.Copy,
                                     scale=1.0 / 16.0)
                rch = spool.tile([1, HW_], F32, tag="recf", bufs=2, name="rcf")
                nc.vector.reciprocal_approx_fast(out=rch[:],
                                                 in_=rs[:, lo:lo + HW_])
                rbh = spool.tile([1, HW_], BF16, tag="rbf", bufs=2, name="rbf")
                nc.gpsimd.tensor_copy(rbh[:], rch[:])
                flush_exp()
                _, pet = st_slot()
                nc.tensor.matmul(pet[:, 0:HW_], wvT, z8h[:],
                                 start=True, stop=True)
                nc.tensor.matmul(pet[:, 512:512 + HW_], ones_row[:], rbh[:],
                                 start=True, stop=True)
                osh = spool.tile([C, HW_], F32, tag="osf", bufs=2, name="osf")
                nc.scalar.copy(osh[:], pet[:, 0:HW_])
                t1h = spool.tile([C, HW_], BF16, tag="t1f", bufs=2, name="t1f")
                nc.vector.tensor_tensor(out=t1h[:], in0=pet[:, 512:512 + HW_],
                                        in1=osh[:], op=ALU.mult)
                cl = bass.ds(ch * CHUNK + lo, HW_)
                nc.gpsimd.tensor_add(resf[:, lo:lo + HW_], t1h[:],
                                     xq_sb[:, cl])
            nc.sync.dma_start(y[:, bass.ts(ch, CHUNK)], resf[:])

        TOT = NCHUNKS * PAIRS
        o3 = ones_db.rearrange("p (b c) -> p b c", b=2)[:, :, 0:1]
        for rep in range(repeats):
            zcur = {}
            for gp in range(TOT + LAGP):
                if rep == 0:
                    for f in extras.get(gp, ()):
                        f()
                s2 = gp - LAGP
                if s2 >= 0:
                    sp = s2 % PAIRS
                    if sp == 0 and "z" in pend:
                        epi_a()
                    if sp == 2 and "z8" in pend:
                        epi_m()
                    if sp == 4 and "pe_t" in pend:
                        epi_b()
                if gp < TOT:
                    ch = gp // PAIRS
                    s = gp % PAIRS
                    sl = bass.ts(ch, CHUNK)
                    i, stp = st_slot()
                    for j in range(2):
                        t = 2 * s + j
                        nc.tensor.matmul(stp[:, 512 * j:512 * j + CHUNK],
                                         k3[:, :, bass.ts(t, 128)],
                                         q3[:, :, sl],
                                         start=True, stop=True, perf_mode=DR)
                    pair_exp(gp, i)
                else:
                    flush_exp()
                if s2 >= 0:
                    ch2 = s2 // PAIRS
                    s = s2 % PAIRS
                    if s == 0:
                        zcur["z"] = ppool.tile([C, CHUNK], F32, tag="z",
                                               name="zpsum")
                        zcur["rs"] = ppool.tile([1, CHUNK], F32, tag="rs",
                                                name="rspsum")
                    ex, qw, qb = ex_tiles.pop(s2)
                    ex3 = ex.rearrange("p (q x) -> p q x",
                                       q=qw)[:, qb:qb + 2, 0:CHUNK]
                    xt3 = xkvT_sb[:, bass.ds(256 * s, 256)].rearrange(
                        "p (b c) -> p b c", b=2)
                    nc.tensor.matmul(zcur["z"][:], xt3, ex3, perf_mode=DR,
                                     start=(s == 0), stop=(s == PAIRS - 1))
                    nc.tensor.matmul(zcur["rs"][:], o3, ex3, perf_mode=DR,
                                     start=(s == 0), stop=(s == PAIRS - 1))
                    if s == PAIRS - 1:
                        pend.update(z=zcur.pop("z"), rs=zcur.pop("rs"),
                                    sl=bass.ts(ch2, CHUNK))
            if rep != repeats - 1:
                epi_a()
                epi_m()
                epi_b()
                tc.strict_bb_all_engine_barrier()
        if "z" in pend:
            pend.pop("sl")
            epi_final(NCHUNKS - 1)

    nc.compile()
    _BUILD_CACHE[key] = nc
    return nc


def _prep_in_maps(x_q, x_kv, Wq, bq, Wk, bk, Wv, bv, gamma):
    bf16 = ml_dtypes.bfloat16
    f8 = ml_dtypes.float8_e4m3
    f32 = np.float32
    x_q = np.asarray(x_q, f32).reshape(C, N)
    x_kv = np.asarray(x_kv, f32).reshape(C, N)
    Wq = np.asarray(Wq, f32)
    bq = np.asarray(bq, f32)
    Wk = np.asarray(Wk, f32)
    bk = np.asarray(bk, f32)
    Wv = np.asarray(Wv, f32)
    bv = np.asarray(bv, f32)
    gamma = float(np.asarray(gamma, f32).reshape(()))

    xkv_f8 = np.ascontiguousarray(x_kv).astype(f8)
    # xkv transposed [m, c] tiled by 128 keys (Z matmul stationary)
    xkvT = np.ascontiguousarray(
        x_kv.reshape(C, MT, 128).transpose(2, 1, 0).reshape(128, MT * C)).astype(f8)
    # bf16 weights blob: Wq^T/4 | 16*Wk^T | gamma*Wv^T
    wbf = np.zeros((C, 2 * RC + C), f32)
    wbf[:, 0:RC] = Wq.T * 0.25
    wbf[:, RC:2 * RC] = Wk.T * 16.0
    wbf[:, 2 * RC:2 * RC + C] = (gamma * Wv).T
    wbf = np.ascontiguousarray(wbf).astype(bf16)
    # biases: col0 = 16*bk tiled at rows 32g+r ; col1/col2 = bq/4 halves
    bias2 = np.zeros((C, 3), f32)
    for g in range(4):
        bias2[32 * g:32 * g + RC, 0] = 16.0 * bk
    bias2[0:8, 1] = 0.25 * bq[0:8]
    bias2[0:8, 2] = 0.25 * bq[8:16]
    resid_bias = (gamma * bv).astype(f32)  # softmax rows sum to 1

    in_maps = []
    for c in range(NCORES):
        xq_slice = np.ascontiguousarray(
            x_q[:, c * NQ:(c + 1) * NQ] + resid_bias[:, None]).astype(bf16)
        in_maps.append({
            "wbf": wbf, "bias2": bias2,
            "xkv_f8": xkv_f8, "xq_bf": xq_slice, "xkvT": xkvT,
        })
    return in_maps


def kernel(x_q, x_kv, Wq, bq, Wk, bk, Wv, bv, gamma):
    nc = build_nc(repeats=1)
    in_maps = _prep_in_maps(x_q, x_kv, Wq, bq, Wk, bk, Wv, bv, gamma)
    res = run_bass_kernel_spmd(nc, in_maps, list(range(NCORES)))
    out = np.concatenate([res.results[c]["y"].astype(np.float32)
                          for c in range(NCORES)], axis=1)
    return out.reshape(1, C, D, H, W).astype(np.float32)


# revision 52
# speedup vs baseline: 2.3193x; 2.3193x over previous
"""CrossAttentionBlock Trainium2 kernel (v2).

Math (reference):
    q = Wq@xq + bq        [RC=16, N]     (per-voxel 1x1x1 conv == channel matmul)
    k = Wk@xkv + bk       [16, N]
    v = Wv@xkv + bv       [C=128, N]
    S = (q^T k) / 4       [N, N]
    P = softmax_rows(S)
    out = v @ P^T         [C, N]
    y = x_q + gamma*out

Kernel strategy (8 NeuronCores, sequence-parallel over the N=13824 query
tokens; each core owns NQ=1728 queries against full K/V):
  * The hard throughput floor is PSUM->SBUF evacuation bandwidth: only the
    Activation and DVE engines can read PSUM, and every exp'd score element
    must cross once (exp is fused into the evacuation op, so exp itself is
    free).  Everything else is arranged to keep that path minimal:
      - v is never materialized: out = (gamma*Wv) @ (xkv @ P^T) reassociated,
        so the big [C,N] v evacuation disappears; Z = xkv @ exp(S^T)
        accumulates in PSUM via the same per-pair matmuls and is evacuated
        once per chunk ([C,432] instead of [C,N]).
      - the k projection packs 3 column-groups of 16 output rows into one
        [128,512] PSUM tile (base partitions 0/32/64), so its evacuation
        runs with full 128-lane utilization; a few SBUF->SBUF DMAs remap to
        the DoubleRow layout afterwards.
  * Scores are built TRANSPOSED (S^T tiles [128 keys x 432 queries]), fp8 +
    DoubleRow everywhere (2 MACs/cell/cycle): k is host-scaled by 16 into
    fp8 weights, so exp applies a 1/16 input scale (ScalarE scale arg /
    folded into the Schraudolph constant on DVE).  No max subtraction
    (|S|<~2 by construction); normalization deferred: Z and a ones-row
    matmul (ones=0.25 folds the Z evac scale) accumulate per chunk, then
    reciprocal + 1->128 broadcast matmul + multiply + residual add.
  * exp alternates ScalarE (true exp, fp8 out) / VectorE (Schraudolph int8
    bit-trick) ~53/47 Bresenham-interleaved.  Inputs land fp8/bf16 (xkv in
    both [c,m] and [m,c] layouts), chunked DMAs so projections overlap the
    loads.  Attention contributes O(1e-4) of the output, so fp8 noise is
    invisible; the residual is bf16 (0.2% of tolerance).
"""

import contextlib

import numpy as np
import ml_dtypes

import concourse.bass as bass
import concourse.mybir as mybir
from concourse import bacc
from concourse.tile import TileContext
from concourse.bass_utils import run_bass_kernel_spmd

F32 = mybir.dt.float32
BF16 = mybir.dt.bfloat16
FP8 = mybir.dt.float8e4
I8 = mybir.dt.int8
AF = mybir.ActivationFunctionType
DR = mybir.MatmulPerfMode.DoubleRow
ALU = mybir.AluOpType

C = 128           # channels
RC = 16           # reduced (q/k) channels
D = H = W = 24
N = D * H * W     # 13824 tokens
NCORES = 8
NQ = N // NCORES  # 1728 queries per core
CHUNK = 432       # query chunk ([128, CHUNK] f32 fits half a PSUM slot)
NCHUNKS = NQ // CHUNK   # 4
MT = N // 128     # 108 key tiles of 128
PAIRS = MT // 2   # 54 key-tile pairs per chunk
LAGP = 6          # Z/rs matmuls trail exp by this many pairs

KTW = 512         # k-projection column width per matmul (one PSUM bank)
KGROUPS = 3       # k output-row groups per PSUM tile (base partitions 0/32/64)
KTILES = N // (KTW * KGROUPS)   # 9

LOG2E = 1.4426950408889634
EXP8_SCALE = 8.0 * LOG2E / 16.0   # e4m3 bit trick, folding the S'=16*S scale
EXP8_BIAS = 56.0 - 0.3            # 7*8 + Schraudolph offset
ACT_FRAC = 0.531                  # ScalarE share of exp ops (1025/(905+1025))


def _act_pattern(n):
    pat, acc = [], 0.0
    for _ in range(n):
        acc += ACT_FRAC
        if acc >= 1.0:
            acc -= 1.0
            pat.append(True)
        else:
            pat.append(False)
    return pat

_BUILD_CACHE: dict = {}


def build_nc(repeats: int = 1):
    """Build + compile the per-core Bass program (SPMD across 8 cores)."""
    key = repeats
    if key in _BUILD_CACHE:
        return _BUILD_CACHE[key]

    nc = bacc.Bacc("TRN2", target_bir_lowering=False, debug=False,
                   num_devices=NCORES)
    wbf = nc.dram_tensor("wbf", [C, 2 * RC + C], BF16, kind="ExternalInput").ap()
    bias2 = nc.dram_tensor("bias2", [C, 3], F32, kind="ExternalInput").ap()
    xkv_f8 = nc.dram_tensor("xkv_f8", [C, N], FP8, kind="ExternalInput").ap()
    xq_bf = nc.dram_tensor("xq_bf", [C, NQ], BF16, kind="ExternalInput").ap()
    xkvT = nc.dram_tensor("xkvT", [C, N], FP8, kind="ExternalInput").ap()
    y = nc.dram_tensor("y", [C, NQ], BF16, kind="ExternalOutput").ap()

    with TileContext(nc) as tc, contextlib.ExitStack() as ctx:
        cpool = ctx.enter_context(tc.tile_pool(name="consts", bufs=1))
        ppool = ctx.enter_context(tc.tile_pool(name="psum", bufs=1, space="PSUM"))
        spool = ctx.enter_context(tc.tile_pool(name="work", bufs=1))

        # ---- resident inputs (issue order == HWDGE order: critical first) --
        KT1 = KTW * KGROUPS          # one k-tile's worth of xkv columns
        wbf_sb = cpool.tile([C, 2 * RC + C], BF16)
        nc.sync.dma_start(wbf_sb[:], wbf[:])
        bias_sb = cpool.tile([C, 3], F32)
        nc.sync.dma_start(bias_sb[:], bias2[:])
        xq_sb = cpool.tile([C, NQ], BF16)
        nc.sync.dma_start(xq_sb[:, 0:CHUNK], xq_bf[:, 0:CHUNK])
        xkv_sb = cpool.tile([C, N], FP8)
        # small chunks first so the k projection starts early; the big tail
        # chunks are issued AFTER the first k remaps so the remap transfers
        # aren't stuck behind them on the (exclusive) DMA engines.
        nc.sync.dma_start(xkv_sb[:, 0:KT1], xkv_f8[:, 0:KT1])
        nc.sync.dma_start(xkv_sb[:, KT1:2 * KT1], xkv_f8[:, KT1:2 * KT1])
        nc.sync.dma_start(xq_sb[:, CHUNK:NQ], xq_bf[:, CHUNK:NQ])
        XH = (N - 2 * KT1) // 2
        nc.sync.dma_start(xkv_sb[:, 2 * KT1:2 * KT1 + XH],
                          xkv_f8[:, 2 * KT1:2 * KT1 + XH])
        nc.sync.dma_start(xkv_sb[:, 2 * KT1 + XH:N], xkv_f8[:, 2 * KT1 + XH:N])
        xkvT_sb = cpool.tile([C, N], FP8)

        wqT = wbf_sb[:, 0:RC]
        wkT = wbf_sb[:, RC:2 * RC]
        wvT = wbf_sb[:, 2 * RC:2 * RC + C]
        bk16 = bias_sb[:, 0:1]
        bq_lo = bias_sb[0:8, 1:2]   # bq[p]/4 on partition p
        bq_hi = bias_sb[0:8, 2:3]   # bq[8+p]/4 on partition p

        ones_db = cpool.tile([C, 32], FP8)
        nc.gpsimd.memset(ones_db[:], 0.0625)   # folds the Z-evac 1/16 scale
        ones_row = cpool.tile([1, C], BF16)  # lhsT for 1->128 broadcast matmul
        nc.gpsimd.memset(ones_row[:], 1.0)
        warm_mv = cpool.tile([1, 512], BF16)
        nc.gpsimd.memset(warm_mv[:], 0.0)

        # PE p-state warmup: keep PE busy early so projection matmuls run at
        # full clock once their inputs land.
        warm_ps = ppool.tile([C, 512], F32, tag="rs", bufs=1, name="warm_ps")
        for _ in range(6):
            nc.tensor.matmul(warm_ps[0:1, :], ones_row[:, 0:1], warm_mv[:],
                             start=True, stop=True)


        def st_slot():
            t = ppool.tile([C, 1024], F32, tag="st", bufs=3, name="stslot")
            return 0, t

        # greedy engine balancing on projected finish time
        eng_t = {"act": 1283.0, "dve": 0.0}  # ACT starts with the table load

        def pick(cost_act, cost_dve):
            if eng_t["act"] + cost_act <= eng_t["dve"] + cost_dve:
                eng_t["act"] += cost_act
                return True
            eng_t["dve"] += cost_dve
            return False

        ex_tiles = {}

        def flush_exp():
            pass

        def pair_exp(gp, stp):
            view = stp.rearrange("p (q x) -> p q x", q=2)[:, :, 0:CHUNK]
            ex = spool.tile([C, 2 * CHUNK], FP8, tag="ex2", bufs=LAGP + 3,
                            name="ex")
            exv = ex.rearrange("p (q x) -> p q x", q=2)
            if pick(905.0, 1025.0):
                nc.scalar.activation(exv, view, AF.Exp, scale=1.0 / 16.0)
            else:
                nc.vector.tensor_scalar(out=exv.bitcast(I8), in0=view,
                                        scalar1=EXP8_SCALE, scalar2=EXP8_BIAS,
                                        op0=ALU.mult, op1=ALU.add)
            ex_tiles[gp] = (ex, 2, 0)

        # ---- projections ---------------------------------------------------
        # k': [128, KTILES*KTW] fp8, partition 32g+r holds 16*k[r] for column
        # group g; evacuations run full-width, then SBUF->SBUF DMAs remap to
        # the DoubleRow layout (o=0 on HWDGE/SP, o=1 on SWDGE/Pool).  q': two
        # 8-row matmuls per chunk-group write the DoubleRow halves side by
        # side in PSUM; 8-lane evacs land straight in q_db layout (no remap).
        # Only k tiles 0-1 and q group 0 run before the attention loop; the
        # rest is interleaved into chunk 0's pair pipeline below.
        k_sb = cpool.tile([C, KTILES * KTW], FP8)
        q_db = cpool.tile([8, 2 * NQ], FP8)
        qdv = q_db.rearrange("p (o g m) -> p o g m", o=2, g=NCHUNKS)
        k_db = cpool.tile([8, 2 * N], FP8)
        kv = k_sb.rearrange("p (t m) -> p t m", t=KTILES)
        kdv = k_db.rearrange("p (o t g m) -> p o t g m", o=2, t=KTILES, g=KGROUPS)

        def q_proj(g):
            flush_exp()
            _, psq = st_slot()
            for o in range(2):
                nc.tensor.matmul(psq[0:8, 512 * o:512 * o + CHUNK],
                                 wqT[:, 8 * o:8 * o + 8],
                                 xq_sb[:, bass.ts(g, CHUNK)],
                                 start=True, stop=True)
            for o, b in ((0, bq_lo), (1, bq_hi)):
                src = psq[0:8, 512 * o:512 * o + CHUNK]
                if pick(545, 575):
                    nc.scalar.activation(qdv[:, o, g, :], src, AF.Identity,
                                         bias=b)
                else:
                    nc.vector.tensor_scalar(out=qdv[:, o, g, :], in0=src,
                                            scalar1=b, scalar2=None,
                                            op0=ALU.add)

        def k_tile(t):
            flush_exp()
            _, psk = st_slot()
            for g in range(KGROUPS):
                lo = (KGROUPS * t + g) * KTW
                nc.tensor.matmul(psk[32 * g:32 * g + RC, 0:KTW],
                                 wkT, xkv_sb[:, lo:lo + KTW],
                                 start=True, stop=True)
            dst = k_sb[:, bass.ts(t, KTW)]
            if pick(612, 658):
                nc.scalar.activation(dst, psk[:, 0:KTW], AF.Identity, bias=bk16)
            else:
                nc.vector.tensor_scalar(out=dst, in0=psk[:, 0:KTW],
                                        scalar1=bk16, scalar2=None, op0=ALU.add)

        def k_remap(t0, t1, pool_frac=False):
            # critical remaps all ride HWDGE (SP); late batches push the o=1
            # half through SWDGE (Pool) to keep HWDGE clear for xkvT loads
            tsl = slice(t0, t1)
            for g in range(KGROUPS):
                nc.sync.dma_start(kdv[:, 0, tsl, g, :],
                                  kv[32 * g:32 * g + 8, tsl, :])
                eng = nc.gpsimd if pool_frac else nc.sync
                eng.dma_start(kdv[:, 1, tsl, g, :],
                              kv[32 * g + 8:32 * g + 16, tsl, :])

        def xkvT_load(qq):
            sl = bass.ts(qq, N // 4)
            nc.sync.dma_start(xkvT_sb[:, sl], xkvT[:, sl])

        k_tile(0)
        q_proj(0)
        k_tile(1)
        k_remap(0, 2, True)
        xkvT_load(0)

        q3 = q_db.rearrange("p (o x) -> p o x", o=2)
        k3 = k_db.rearrange("p (o x) -> p o x", o=2)
        # remaining projection work, interleaved at chunk-0 pair slots
        extras = {
            0: [lambda: k_tile(2)],
            2: [lambda: k_tile(3)],
            4: [lambda: k_remap(2, 4, True)],
            5: [lambda: xkvT_load(1)],
            6: [lambda: k_tile(4)],
            8: [lambda: k_tile(5)],
            10: [lambda: k_tile(6)],
            12: [lambda: k_tile(7)],
            14: [lambda: k_tile(8)],
            15: [lambda: q_proj(1), lambda: k_remap(4, KTILES, True)],
            16: [lambda: xkvT_load(2)],
            17: [lambda: q_proj(2)],
            18: [lambda: xkvT_load(3)],
            19: [lambda: q_proj(3)],
        }

        # ---- attention main loop ------------------------------------------
        # The per-chunk epilogue is deferred into the NEXT chunk's pipeline.
        pend = {}

        def epi_a():
            eng_t["act"] += 545.0
            eng_t["dve"] += 575.0
            # evacuate Z (frees its bank for the next chunk) + reciprocal
            pend["z8"] = z8 = spool.tile([C, CHUNK], FP8, tag="z8", bufs=2,
                                         name="z8")
            nc.scalar.activation(z8[:], pend.pop("z")[:], AF.Copy,
                                 scale=1.0 / 16.0)
            recip = spool.tile([1, CHUNK], F32, tag="recip", bufs=2)
            nc.vector.reciprocal_approx_fast(out=recip[:], in_=pend.pop("rs")[:])
            pend["recip_bf"] = recip_bf = spool.tile([1, CHUNK], BF16,
                                                     tag="recipb", bufs=2,
                                                     name="recip_bf")
            nc.gpsimd.tensor_copy(recip_bf[:], recip[:])

        def epi_m():
            # borrow one arena slot: outu (cols 0:CHUNK) + bcp (512:512+CHUNK)
            eng_t["act"] += 545.0
            flush_exp()
            _, pet = st_slot()
            pend["pe_t"] = pet
            nc.tensor.matmul(pet[:, 0:CHUNK], wvT, pend.pop("z8")[:],
                             start=True, stop=True)
            nc.tensor.matmul(pet[:, 512:512 + CHUNK], ones_row[:],
                             pend.pop("recip_bf")[:], start=True, stop=True)
            pend["outu_s"] = outu_s = spool.tile([C, CHUNK], F32, tag="outu_s",
                                                 bufs=2, name="outu_s")
            nc.scalar.copy(outu_s[:], pet[:, 0:CHUNK])

        def epi_b():
            eng_t["dve"] += 575.0
            sl = pend.pop("sl")
            pet = pend.pop("pe_t")
            t1 = spool.tile([C, CHUNK], BF16, tag="t1", bufs=2)
            nc.vector.tensor_tensor(out=t1[:], in0=pet[:, 512:512 + CHUNK],
                                    in1=pend.pop("outu_s")[:], op=ALU.mult)
            res = spool.tile([C, CHUNK], BF16, tag="res", bufs=2)
            nc.gpsimd.tensor_add(res[:], t1[:], xq_sb[:, sl])
            nc.sync.dma_start(y[:, sl], res[:])

        def epi_final(ch):
            # tail-latency version: two column halves pipelined across
            # engines, one y DMA at the end
            z, rs = pend.pop("z"), pend.pop("rs")
            HW_ = CHUNK // 2
            resf = spool.tile([C, CHUNK], BF16, tag="resf", bufs=1, name="rsf")
            for h in range(2):
                lo = h * HW_
                z8h = spool.tile([C, HW_], FP8, tag="z8f", bufs=2, name="z8f")
                nc.scalar.activation(z8h[:], z[:, lo:lo + HW_], AF.Copy,
                                     scale=1.0 / 16.0)
                rch = spool.tile([1, HW_], F32, tag="recf", bufs=2, name="rcf")
                nc.vector.reciprocal_approx_fast(out=rch[:],
                                                 in_=rs[:, lo:lo + HW_])
                rbh = spool.tile([1, HW_], BF16, tag="rbf", bufs=2, name="rbf")
                nc.gpsimd.tensor_copy(rbh[:], rch[:])
                flush_exp()
                _, pet = st_slot()
                nc.tensor.matmul(pet[:, 0:HW_], wvT, z8h[:],
                                 start=True, stop=True)
                nc.tensor.matmul(pet[:, 512:512 + HW_], ones_row[:], rbh[:],
                                 start=True, stop=True)
                osh = spool.tile([C, HW_], F32, tag="osf", bufs=2, name="osf")
                nc.scalar.copy(osh[:], pet[:, 0:HW_])
                t1h = spool.tile([C, HW_], BF16, tag="t1f", bufs=2, name="t1f")
                nc.vector.tensor_tensor(out=t1h[:], in0=pet[:, 512:512 + HW_],
                                        in1=osh[:], op=ALU.mult)
                cl = bass.ds(ch * CHUNK + lo, HW_)
                nc.gpsimd.tensor_add(resf[:, lo:lo + HW_], t1h[:],
                                     xq_sb[:, cl])
            nc.sync.dma_start(y[:, bass.ts(ch, CHUNK)], resf[:])

        TOT = NCHUNKS * PAIRS
        o3 = ones_db.rearrange("p (b c) -> p b c", b=2)[:, :, 0:1]
        for rep in range(repeats):
            zcur = {}
            for gp in range(TOT + LAGP):
                if rep == 0:
                    for f in extras.get(gp, ()):
                        f()
                s2 = gp - LAGP
                if s2 >= 0:
                    sp = s2 % PAIRS
                    if sp == 0 and "z" in pend:
                        epi_a()
                    if sp == 2 and "z8" in pend:
                        epi_m()
                    if sp == 4 and "pe_t" in pend:
                        epi_b()
                if gp < TOT:
                    ch = gp // PAIRS
                    s = gp % PAIRS
                    sl = bass.ts(ch, CHUNK)
                    i, stp = st_slot()
                    for j in range(2):
                        t = 2 * s + j
                        nc.tensor.matmul(stp[:, 512 * j:512 * j + CHUNK],
                                         k3[:, :, bass.ts(t, 128)],
                                         q3[:, :, sl],
                                         start=True, stop=True, perf_mode=DR)
                    pair_exp(gp, stp)
                if s2 >= 0:
                    ch2 = s2 // PAIRS
                    s = s2 % PAIRS
                    if s == 0:
                        zcur["z"] = ppool.tile([C, CHUNK], F32, tag="z",
                                               name="zpsum")
                        zcur["rs"] = ppool.tile([1, CHUNK], F32, tag="rs",
                                                name="rspsum")
                    ex, qw, qb = ex_tiles.pop(s2)
                    ex3 = ex.rearrange("p (q x) -> p q x",
                                       q=qw)[:, qb:qb + 2, 0:CHUNK]
                    xt3 = xkvT_sb[:, bass.ds(256 * s, 256)].rearrange(
                        "p (b c) -> p b c", b=2)
                    nc.tensor.matmul(zcur["z"][:], xt3, ex3, perf_mode=DR,
                                     start=(s == 0), stop=(s == PAIRS - 1))
                    nc.tensor.matmul(zcur["rs"][:], o3, ex3, perf_mode=DR,
                                     start=(s == 0), stop=(s == PAIRS - 1))
                    if s == PAIRS - 1:
                        pend.update(z=zcur.pop("z"), rs=zcur.pop("rs"),
                                    sl=bass.ts(ch2, CHUNK))
            if rep != repeats - 1:
                epi_a()
                epi_m()
                epi_b()
                tc.strict_bb_all_engine_barrier()
        if "z" in pend:
            pend.pop("sl")
            epi_final(NCHUNKS - 1)

    nc.compile()
    _BUILD_CACHE[key] = nc
    return nc


def _prep_in_maps(x_q, x_kv, Wq, bq, Wk, bk, Wv, bv, gamma):
    bf16 = ml_dtypes.bfloat16
    f8 = ml_dtypes.float8_e4m3
    f32 = np.float32
    x_q = np.asarray(x_q, f32).reshape(C, N)
    x_kv = np.asarray(x_kv, f32).reshape(C, N)
    Wq = np.asarray(Wq, f32)
    bq = np.asarray(bq, f32)
    Wk = np.asarray(Wk, f32)
    bk = np.asarray(bk, f32)
    Wv = np.asarray(Wv, f32)
    bv = np.asarray(bv, f32)
    gamma = float(np.asarray(gamma, f32).reshape(()))

    xkv_f8 = np.ascontiguousarray(x_kv).astype(f8)
    # xkv transposed [m, c] tiled by 128 keys (Z matmul stationary)
    xkvT = np.ascontiguousarray(
        x_kv.reshape(C, MT, 128).transpose(2, 1, 0).reshape(128, MT * C)).astype(f8)
    # bf16 weights blob: Wq^T/4 | 16*Wk^T | gamma*Wv^T
    wbf = np.zeros((C, 2 * RC + C), f32)
    wbf[:, 0:RC] = Wq.T * 0.25
    wbf[:, RC:2 * RC] = Wk.T * 16.0
    wbf[:, 2 * RC:2 * RC + C] = (gamma * Wv).T
    wbf = np.ascontiguousarray(wbf).astype(bf16)
    # biases: col0 = 16*bk tiled at rows 32g+r ; col1/col2 = bq/4 halves
    bias2 = np.zeros((C, 3), f32)
    for g in range(4):
        bias2[32 * g:32 * g + RC, 0] = 16.0 * bk
    bias2[0:8, 1] = 0.25 * bq[0:8]
    bias2[0:8, 2] = 0.25 * bq[8:16]
    resid_bias = (gamma * bv).astype(f32)  # softmax rows sum to 1

    in_maps = []
    for c in range(NCORES):
        xq_slice = np.ascontiguousarray(
            x_q[:, c * NQ:(c + 1) * NQ] + resid_bias[:, None]).astype(bf16)
        in_maps.append({
            "wbf": wbf, "bias2": bias2,
            "xkv_f8": xkv_f8, "xq_bf": xq_slice, "xkvT": xkvT,
        })
    return in_maps


def kernel(x_q, x_kv, Wq, bq, Wk, bk, Wv, bv, gamma):
    nc = build_nc(repeats=1)
    in_maps = _prep_in_maps(x_q, x_kv, Wq, bq, Wk, bk, Wv, bv, gamma)
    res = run_bass_kernel_spmd(nc, in_maps, list(range(NCORES)))
    out = np.concatenate([res.results[c]["y"].astype(np.float32)
                          for c in range(NCORES)], axis=1)
    return out.reshape(1, C, D, H, W).astype(np.float32)


# revision 54
# speedup vs baseline: 2.3267x; 1.0032x over previous
"""CrossAttentionBlock Trainium2 kernel (v2).

Math (reference):
    q = Wq@xq + bq        [RC=16, N]     (per-voxel 1x1x1 conv == channel matmul)
    k = Wk@xkv + bk       [16, N]
    v = Wv@xkv + bv       [C=128, N]
    S = (q^T k) / 4       [N, N]
    P = softmax_rows(S)
    out = v @ P^T         [C, N]
    y = x_q + gamma*out

Kernel strategy (8 NeuronCores, sequence-parallel over the N=13824 query
tokens; each core owns NQ=1728 queries against full K/V):
  * The hard throughput floor is PSUM->SBUF evacuation bandwidth: only the
    Activation and DVE engines can read PSUM, and every exp'd score element
    must cross once (exp is fused into the evacuation op, so exp itself is
    free).  Everything else is arranged to keep that path minimal:
      - v is never materialized: out = (gamma*Wv) @ (xkv @ P^T) reassociated,
        so the big [C,N] v evacuation disappears; Z = xkv @ exp(S^T)
        accumulates in PSUM via the same per-pair matmuls and is evacuated
        once per chunk ([C,432] instead of [C,N]).
      - the k projection packs 3 column-groups of 16 output rows into one
        [128,512] PSUM tile (base partitions 0/32/64), so its evacuation
        runs with full 128-lane utilization; a few SBUF->SBUF DMAs remap to
        the DoubleRow layout afterwards.
  * Scores are built TRANSPOSED (S^T tiles [128 keys x 432 queries]), fp8 +
    DoubleRow everywhere (2 MACs/cell/cycle): k is host-scaled by 16 into
    fp8 weights, so exp applies a 1/16 input scale (ScalarE scale arg /
    folded into the Schraudolph constant on DVE).  No max subtraction
    (|S|<~2 by construction); normalization deferred: Z and a ones-row
    matmul (ones=0.25 folds the Z evac scale) accumulate per chunk, then
    reciprocal + 1->128 broadcast matmul + multiply + residual add.
  * exp alternates ScalarE (true exp, fp8 out) / VectorE (Schraudolph int8
    bit-trick) ~53/47 Bresenham-interleaved.  Inputs land fp8/bf16 (xkv in
    both [c,m] and [m,c] layouts), chunked DMAs so projections overlap the
    loads.  Attention contributes O(1e-4) of the output, so fp8 noise is
    invisible; the residual is bf16 (0.2% of tolerance).
"""

import contextlib

import numpy as np
import ml_dtypes

import concourse.bass as bass
import concourse.mybir as mybir
from concourse import bacc
from concourse.tile import TileContext
from concourse.bass_utils import run_bass_kernel_spmd

F32 = mybir.dt.float32
BF16 = mybir.dt.bfloat16
FP8 = mybir.dt.float8e4
I8 = mybir.dt.int8
AF = mybir.ActivationFunctionType
DR = mybir.MatmulPerfMode.DoubleRow
ALU = mybir.AluOpType

C = 128           # channels
RC = 16           # reduced (q/k) channels
D = H = W = 24
N = D * H * W     # 13824 tokens
NCORES = 8
NQ = N // NCORES  # 1728 queries per core
CHUNK = 432       # query chunk ([128, CHUNK] f32 fits half a PSUM slot)
NCHUNKS = NQ // CHUNK   # 4
MT = N // 128     # 108 key tiles of 128
PAIRS = MT // 2   # 54 key-tile pairs per chunk
LAGP = 6          # Z/rs matmuls trail exp by this many pairs

KTW = 512         # k-projection column width per matmul (one PSUM bank)
KGROUPS = 3       # k output-row groups per PSUM tile (base partitions 0/32/64)
KTILES = N // (KTW * KGROUPS)   # 9

LOG2E = 1.4426950408889634
EXP8_SCALE = 8.0 * LOG2E / 16.0   # e4m3 bit trick, folding the S'=16*S scale
EXP8_BIAS = 56.0 - 0.3            # 7*8 + Schraudolph offset
ACT_FRAC = 0.531                  # ScalarE share of exp ops (1025/(905+1025))


def _act_pattern(n):
    pat, acc = [], 0.0
    for _ in range(n):
        acc += ACT_FRAC
        if acc >= 1.0:
            acc -= 1.0
            pat.append(True)
        else:
            pat.append(False)
    return pat

_BUILD_CACHE: dict = {}


def build_nc(repeats: int = 1):
    """Build + compile the per-core Bass program (SPMD across 8 cores)."""
    key = repeats
    if key in _BUILD_CACHE:
        return _BUILD_CACHE[key]

    nc = bacc.Bacc("TRN2", target_bir_lowering=False, debug=False,
                   num_devices=NCORES)
    wbf = nc.dram_tensor("wbf", [C, 2 * RC + C], BF16, kind="ExternalInput").ap()
    bias2 = nc.dram_tensor("bias2", [C, 3], F32, kind="ExternalInput").ap()
    xkv_f8 = nc.dram_tensor("xkv_f8", [C, N], FP8, kind="ExternalInput").ap()
    xq_bf = nc.dram_tensor("xq_bf", [C, NQ], BF16, kind="ExternalInput").ap()
    xkvT = nc.dram_tensor("xkvT", [C, N], FP8, kind="ExternalInput").ap()
    y = nc.dram_tensor("y", [C, NQ], BF16, kind="ExternalOutput").ap()

    with TileContext(nc) as tc, contextlib.ExitStack() as ctx:
        cpool = ctx.enter_context(tc.tile_pool(name="consts", bufs=1))
        ppool = ctx.enter_context(tc.tile_pool(name="psum", bufs=1, space="PSUM"))
        spool = ctx.enter_context(tc.tile_pool(name="work", bufs=1))

        # ---- resident inputs (issue order == HWDGE order: critical first) --
        KT1 = KTW * KGROUPS          # one k-tile's worth of xkv columns
        xkv_sb = cpool.tile([C, N], FP8)
        # first k-tile's columns + chunk-0 queries lead; weights right behind
        nc.sync.dma_start(xkv_sb[:, 0:KT1], xkv_f8[:, 0:KT1])
        xq_sb = cpool.tile([C, NQ], BF16)
        nc.sync.dma_start(xq_sb[:, 0:CHUNK], xq_bf[:, 0:CHUNK])
        wbf_sb = cpool.tile([C, 2 * RC + C], BF16)
        nc.sync.dma_start(wbf_sb[:], wbf[:])
        bias_sb = cpool.tile([C, 3], F32)
        nc.sync.dma_start(bias_sb[:], bias2[:])
        nc.sync.dma_start(xkv_sb[:, KT1:2 * KT1], xkv_f8[:, KT1:2 * KT1])
        nc.sync.dma_start(xq_sb[:, CHUNK:NQ], xq_bf[:, CHUNK:NQ])
        XH = (N - 2 * KT1) // 2
        nc.sync.dma_start(xkv_sb[:, 2 * KT1:2 * KT1 + XH],
                          xkv_f8[:, 2 * KT1:2 * KT1 + XH])
        nc.sync.dma_start(xkv_sb[:, 2 * KT1 + XH:N], xkv_f8[:, 2 * KT1 + XH:N])
        xkvT_sb = cpool.tile([C, N], FP8)

        wqT = wbf_sb[:, 0:RC]
        wkT = wbf_sb[:, RC:2 * RC]
        wvT = wbf_sb[:, 2 * RC:2 * RC + C]
        bk16 = bias_sb[:, 0:1]
        bq_lo = bias_sb[0:8, 1:2]   # bq[p]/4 on partition p
        bq_hi = bias_sb[0:8, 2:3]   # bq[8+p]/4 on partition p

        ones_db = cpool.tile([C, 32], FP8)
        nc.gpsimd.memset(ones_db[:], 0.0625)   # folds the Z-evac 1/16 scale
        ones_row = cpool.tile([1, C], BF16)  # lhsT for 1->128 broadcast matmul
        nc.gpsimd.memset(ones_row[:], 1.0)
        warm_mv = cpool.tile([1, 512], BF16)
        nc.gpsimd.memset(warm_mv[:], 0.0)

        # PE p-state warmup: keep PE busy early so projection matmuls run at
        # full clock once their inputs land.
        warm_ps = ppool.tile([C, 512], F32, tag="rs", bufs=1, name="warm_ps")
        for _ in range(6):
            nc.tensor.matmul(warm_ps[0:1, :], ones_row[:, 0:1], warm_mv[:],
                             start=True, stop=True)


        def st_slot():
            t = ppool.tile([C, 1024], F32, tag="st", bufs=3, name="stslot")
            return 0, t

        # greedy engine balancing on projected finish time
        eng_t = {"act": 1283.0, "dve": 0.0}  # ACT starts with the table load

        def pick(cost_act, cost_dve):
            if eng_t["act"] + cost_act <= eng_t["dve"] + cost_dve:
                eng_t["act"] += cost_act
                return True
            eng_t["dve"] += cost_dve
            return False

        ex_tiles = {}

        def flush_exp():
            pass

        def pair_exp(gp, stp):
            view = stp.rearrange("p (q x) -> p q x", q=2)[:, :, 0:CHUNK]
            ex = spool.tile([C, 2 * CHUNK], FP8, tag="ex2", bufs=LAGP + 3,
                            name="ex")
            exv = ex.rearrange("p (q x) -> p q x", q=2)
            if pick(905.0, 1025.0):
                nc.scalar.activation(exv, view, AF.Exp, scale=1.0 / 16.0)
            else:
                nc.vector.tensor_scalar(out=exv.bitcast(I8), in0=view,
                                        scalar1=EXP8_SCALE, scalar2=EXP8_BIAS,
                                        op0=ALU.mult, op1=ALU.add)
            ex_tiles[gp] = (ex, 2, 0)

        # ---- projections ---------------------------------------------------
        # k': [128, KTILES*KTW] fp8, partition 32g+r holds 16*k[r] for column
        # group g; evacuations run full-width, then SBUF->SBUF DMAs remap to
        # the DoubleRow layout (o=0 on HWDGE/SP, o=1 on SWDGE/Pool).  q': two
        # 8-row matmuls per chunk-group write the DoubleRow halves side by
        # side in PSUM; 8-lane evacs land straight in q_db layout (no remap).
        # Only k tiles 0-1 and q group 0 run before the attention loop; the
        # rest is interleaved into chunk 0's pair pipeline below.
        k_sb = cpool.tile([C, KTILES * KTW], FP8)
        q_db = cpool.tile([8, 2 * NQ], FP8)
        qdv = q_db.rearrange("p (o g m) -> p o g m", o=2, g=NCHUNKS)
        k_db = cpool.tile([8, 2 * N], FP8)
        kv = k_sb.rearrange("p (t m) -> p t m", t=KTILES)
        kdv = k_db.rearrange("p (o t g m) -> p o t g m", o=2, t=KTILES, g=KGROUPS)

        def q_proj(g):
            flush_exp()
            _, psq = st_slot()
            for o in range(2):
                nc.tensor.matmul(psq[0:8, 512 * o:512 * o + CHUNK],
                                 wqT[:, 8 * o:8 * o + 8],
                                 xq_sb[:, bass.ts(g, CHUNK)],
                                 start=True, stop=True)
            for o, b in ((0, bq_lo), (1, bq_hi)):
                src = psq[0:8, 512 * o:512 * o + CHUNK]
                if pick(545, 575):
                    nc.scalar.activation(qdv[:, o, g, :], src, AF.Identity,
                                         bias=b)
                else:
                    nc.vector.tensor_scalar(out=qdv[:, o, g, :], in0=src,
                                            scalar1=b, scalar2=None,
                                            op0=ALU.add)

        def k_tile(t):
            flush_exp()
            _, psk = st_slot()
            for g in range(KGROUPS):
                lo = (KGROUPS * t + g) * KTW
                nc.tensor.matmul(psk[32 * g:32 * g + RC, 0:KTW],
                                 wkT, xkv_sb[:, lo:lo + KTW],
                                 start=True, stop=True)
            dst = k_sb[:, bass.ts(t, KTW)]
            if pick(612, 658):
                nc.scalar.activation(dst, psk[:, 0:KTW], AF.Identity, bias=bk16)
            else:
                nc.vector.tensor_scalar(out=dst, in0=psk[:, 0:KTW],
                                        scalar1=bk16, scalar2=None, op0=ALU.add)

        def k_remap(t0, t1, pool_frac=False):
            # critical remaps all ride HWDGE (SP); late batches push the o=1
            # half through SWDGE (Pool) to keep HWDGE clear for xkvT loads
            tsl = slice(t0, t1)
            for g in range(KGROUPS):
                nc.sync.dma_start(kdv[:, 0, tsl, g, :],
                                  kv[32 * g:32 * g + 8, tsl, :])
                eng = nc.gpsimd if pool_frac else nc.sync
                eng.dma_start(kdv[:, 1, tsl, g, :],
                              kv[32 * g + 8:32 * g + 16, tsl, :])

        def xkvT_load(qq):
            sl = bass.ts(qq, N // 4)
            nc.sync.dma_start(xkvT_sb[:, sl], xkvT[:, sl])

        k_tile(0)
        q_proj(0)
        k_tile(1)
        k_remap(0, 2, True)
        xkvT_load(0)

        q3 = q_db.rearrange("p (o x) -> p o x", o=2)
        k3 = k_db.rearrange("p (o x) -> p o x", o=2)
        # remaining projection work, interleaved at chunk-0 pair slots
        extras = {
            0: [lambda: k_tile(2)],
            2: [lambda: k_tile(3)],
            4: [lambda: k_remap(2, 4, True)],
            5: [lambda: xkvT_load(1)],
            6: [lambda: k_tile(4)],
            8: [lambda: k_tile(5)],
            10: [lambda: k_tile(6)],
            12: [lambda: k_tile(7)],
            14: [lambda: k_tile(8)],
            15: [lambda: q_proj(1), lambda: k_remap(4, KTILES, True)],
            16: [lambda: xkvT_load(2)],
            17: [lambda: q_proj(2)],
            18: [lambda: xkvT_load(3)],
            19: [lambda: q_proj(3)],
        }

        # ---- attention main loop ------------------------------------------
        # The per-chunk epilogue is deferred into the NEXT chunk's pipeline.
        pend = {}

        def epi_a():
            eng_t["act"] += 545.0
            eng_t["dve"] += 575.0
            # evacuate Z (frees its bank for the next chunk) + reciprocal
            pend["z8"] = z8 = spool.tile([C, CHUNK], FP8, tag="z8", bufs=2,
                                         name="z8")
            nc.scalar.activation(z8[:], pend.pop("z")[:], AF.Copy,
                                 scale=1.0 / 16.0)
            recip = spool.tile([1, CHUNK], F32, tag="recip", bufs=2)
            nc.vector.reciprocal_approx_fast(out=recip[:], in_=pend.pop("rs")[:])
            pend["recip_bf"] = recip_bf = spool.tile([1, CHUNK], BF16,
                                                     tag="recipb", bufs=2,
                                                     name="recip_bf")
            nc.gpsimd.tensor_copy(recip_bf[:], recip[:])

        def epi_m():
            # borrow one arena slot: outu (cols 0:CHUNK) + bcp (512:512+CHUNK)
            eng_t["act"] += 545.0
            flush_exp()
            _, pet = st_slot()
            pend["pe_t"] = pet
            nc.tensor.matmul(pet[:, 0:CHUNK], wvT, pend.pop("z8")[:],
                             start=True, stop=True)
            nc.tensor.matmul(pet[:, 512:512 + CHUNK], ones_row[:],
                             pend.pop("recip_bf")[:], start=True, stop=True)
            pend["outu_s"] = outu_s = spool.tile([C, CHUNK], F32, tag="outu_s",
                                                 bufs=2, name="outu_s")
            nc.scalar.copy(outu_s[:], pet[:, 0:CHUNK])

        def epi_b():
            eng_t["dve"] += 575.0
            sl = pend.pop("sl")
            pet = pend.pop("pe_t")
            t1 = spool.tile([C, CHUNK], BF16, tag="t1", bufs=2)
            nc.vector.tensor_tensor(out=t1[:], in0=pet[:, 512:512 + CHUNK],
                                    in1=pend.pop("outu_s")[:], op=ALU.mult)
            res = spool.tile([C, CHUNK], BF16, tag="res", bufs=2)
            nc.gpsimd.tensor_add(res[:], t1[:], xq_sb[:, sl])
            nc.sync.dma_start(y[:, sl], res[:])

        def epi_final(ch):
            # tail-latency version: two column halves pipelined across
            # engines, one y DMA at the end
            z, rs = pend.pop("z"), pend.pop("rs")
            HW_ = CHUNK // 2
            resf = spool.tile([C, CHUNK], BF16, tag="resf", bufs=1, name="rsf")
            for h in range(2):
                lo = h * HW_
                z8h = spool.tile([C, HW_], FP8, tag="z8f", bufs=2, name="z8f")
                nc.scalar.activation(z8h[:], z[:, lo:lo + HW_], AF.Copy,
                                     scale=1.0 / 16.0)
                rch = spool.tile([1, HW_], F32, tag="recf", bufs=2, name="rcf")
                nc.vector.reciprocal_approx_fast(out=rch[:],
                                                 in_=rs[:, lo:lo + HW_])
                rbh = spool.tile([1, HW_], BF16, tag="rbf", bufs=2, name="rbf")
                nc.gpsimd.tensor_copy(rbh[:], rch[:])
                flush_exp()
                _, pet = st_slot()
                nc.tensor.matmul(pet[:, 0:HW_], wvT, z8h[:],
                                 start=True, stop=True)
                nc.tensor.matmul(pet[:, 512:512 + HW_], ones_row[:], rbh[:],
                                 start=True, stop=True)
                osh = spool.tile([C, HW_], F32, tag="osf", bufs=2, name="osf")
                nc.scalar.copy(osh[:], pet[:, 0:HW_])
                t1h = spool.tile([C, HW_], BF16, tag="t1f", bufs=2, name="t1f")
                nc.vector.tensor_tensor(out=t1h[:], in0=pet[:, 512:512 + HW_],
                                        in1=osh[:], op=ALU.mult)
                cl = bass.ds(ch * CHUNK + lo, HW_)
                nc.vector.tensor_tensor(out=resf[:, lo:lo + HW_], in0=t1h[:],
                                        in1=xq_sb[:, cl], op=ALU.add)
            nc.sync.dma_start(y[:, bass.ts(ch, CHUNK)], resf[:])

        TOT = NCHUNKS * PAIRS
        o3 = ones_db.rearrange("p (b c) -> p b c", b=2)[:, :, 0:1]
        for rep in range(repeats):
            zcur = {}
            for gp in range(TOT + LAGP):
                if rep == 0:
                    for f in extras.get(gp, ()):
                        f()
                s2 = gp - LAGP
                if s2 >= 0:
                    sp = s2 % PAIRS
                    if sp == 0 and "z" in pend:
                        epi_a()
                    if sp == 2 and "z8" in pend:
                        epi_m()
                    if sp == 4 and "pe_t" in pend:
                        epi_b()
                if gp < TOT:
                    ch = gp // PAIRS
                    s = gp % PAIRS
                    sl = bass.ts(ch, CHUNK)
                    i, stp = st_slot()
                    for j in range(2):
                        t = 2 * s + j
                        nc.tensor.matmul(stp[:, 512 * j:512 * j + CHUNK],
                                         k3[:, :, bass.ts(t, 128)],
                                         q3[:, :, sl],
                                         start=True, stop=True, perf_mode=DR)
                    pair_exp(gp, stp)
                if s2 >= 0:
                    ch2 = s2 // PAIRS
                    s = s2 % PAIRS
                    if s == 0:
                        zcur["z"] = ppool.tile([C, CHUNK], F32, tag="z",
                                               name="zpsum")
                        zcur["rs"] = ppool.tile([1, CHUNK], F32, tag="rs",
                                                name="rspsum")
                    ex, qw, qb = ex_tiles.pop(s2)
                    ex3 = ex.rearrange("p (q x) -> p q x",
                                       q=qw)[:, qb:qb + 2, 0:CHUNK]
                    xt3 = xkvT_sb[:, bass.ds(256 * s, 256)].rearrange(
                        "p (b c) -> p b c", b=2)
                    nc.tensor.matmul(zcur["z"][:], xt3, ex3, perf_mode=DR,
                                     start=(s == 0), stop=(s == PAIRS - 1))
                    nc.tensor.matmul(zcur["rs"][:], o3, ex3, perf_mode=DR,
                                     start=(s == 0), stop=(s == PAIRS - 1))
                    if s == PAIRS - 1:
                        pend.update(z=zcur.pop("z"), rs=zcur.pop("rs"),
                                    sl=bass.ts(ch2, CHUNK))
            if rep != repeats - 1:
                epi_a()
                epi_m()
                epi_b()
                tc.strict_bb_all_engine_barrier()
        if "z" in pend:
            pend.pop("sl")
            epi_final(NCHUNKS - 1)

    nc.compile()
    _BUILD_CACHE[key] = nc
    return nc


def _prep_in_maps(x_q, x_kv, Wq, bq, Wk, bk, Wv, bv, gamma):
    bf16 = ml_dtypes.bfloat16
    f8 = ml_dtypes.float8_e4m3
    f32 = np.float32
    x_q = np.asarray(x_q, f32).reshape(C, N)
    x_kv = np.asarray(x_kv, f32).reshape(C, N)
    Wq = np.asarray(Wq, f32)
    bq = np.asarray(bq, f32)
    Wk = np.asarray(Wk, f32)
    bk = np.asarray(bk, f32)
    Wv = np.asarray(Wv, f32)
    bv = np.asarray(bv, f32)
    gamma = float(np.asarray(gamma, f32).reshape(()))

    xkv_f8 = np.ascontiguousarray(x_kv).astype(f8)
    # xkv transposed [m, c] tiled by 128 keys (Z matmul stationary)
    xkvT = np.ascontiguousarray(
        x_kv.reshape(C, MT, 128).transpose(2, 1, 0).reshape(128, MT * C)).astype(f8)
    # bf16 weights blob: Wq^T/4 | 16*Wk^T | gamma*Wv^T
    wbf = np.zeros((C, 2 * RC + C), f32)
    wbf[:, 0:RC] = Wq.T * 0.25
    wbf[:, RC:2 * RC] = Wk.T * 16.0
    wbf[:, 2 * RC:2 * RC + C] = (gamma * Wv).T
    wbf = np.ascontiguousarray(wbf).astype(bf16)
    # biases: col0 = 16*bk tiled at rows 32g+r ; col1/col2 = bq/4 halves
    bias2 = np.zeros((C, 3), f32)
    for g in range(4):
        bias2[32 * g:32 * g + RC, 0] = 16.0 * bk
    bias2[0:8, 1] = 0.25 * bq[0:8]
    bias2[0:8, 2] = 0.25 * bq[8:16]
    resid_bias = (gamma * bv).astype(f32)  # softmax rows sum to 1

    in_maps = []
    for c in range(NCORES):
        xq_slice = np.ascontiguousarray(
            x_q[:, c * NQ:(c + 1) * NQ] + resid_bias[:, None]).astype(bf16)
        in_maps.append({
            "wbf": wbf, "bias2": bias2,
            "xkv_f8": xkv_f8, "xq_bf": xq_slice, "xkvT": xkvT,
        })
    return in_maps


def kernel(x_q, x_kv, Wq, bq, Wk, bk, Wv, bv, gamma):
    nc = build_nc(repeats=1)
    in_maps = _prep_in_maps(x_q, x_kv, Wq, bq, Wk, bk, Wv, bv, gamma)
    res = run_bass_kernel_spmd(nc, in_maps, list(range(NCORES)))
    out = np.concatenate([res.results[c]["y"].astype(np.float32)
                          for c in range(NCORES)], axis=1)
    return out.reshape(1, C, D, H, W).astype(np.float32)


# revision 57
# speedup vs baseline: 2.3362x; 1.0040x over previous
"""CrossAttentionBlock Trainium2 kernel (v2).

Math (reference):
    q = Wq@xq + bq        [RC=16, N]     (per-voxel 1x1x1 conv == channel matmul)
    k = Wk@xkv + bk       [16, N]
    v = Wv@xkv + bv       [C=128, N]
    S = (q^T k) / 4       [N, N]
    P = softmax_rows(S)
    out = v @ P^T         [C, N]
    y = x_q + gamma*out

Kernel strategy (8 NeuronCores, sequence-parallel over the N=13824 query
tokens; each core owns NQ=1728 queries against full K/V):
  * The hard throughput floor is PSUM->SBUF evacuation bandwidth: only the
    Activation and DVE engines can read PSUM, and every exp'd score element
    must cross once (exp is fused into the evacuation op, so exp itself is
    free).  Everything else is arranged to keep that path minimal:
      - v is never materialized: out = (gamma*Wv) @ (xkv @ P^T) reassociated,
        so the big [C,N] v evacuation disappears; Z = xkv @ exp(S^T)
        accumulates in PSUM via the same per-pair matmuls and is evacuated
        once per chunk ([C,432] instead of [C,N]).
      - the k projection packs 3 column-groups of 16 output rows into one
        [128,512] PSUM tile (base partitions 0/32/64), so its evacuation
        runs with full 128-lane utilization; a few SBUF->SBUF DMAs remap to
        the DoubleRow layout afterwards.
  * Scores are built TRANSPOSED (S^T tiles [128 keys x 432 queries]), fp8 +
    DoubleRow everywhere (2 MACs/cell/cycle): k is host-scaled by 16 into
    fp8 weights, so exp applies a 1/16 input scale (ScalarE scale arg /
    folded into the Schraudolph constant on DVE).  No max subtraction
    (|S|<~2 by construction); normalization deferred: Z and a ones-row
    matmul (ones=0.25 folds the Z evac scale) accumulate per chunk, then
    reciprocal + 1->128 broadcast matmul + multiply + residual add.
  * exp alternates ScalarE (true exp, fp8 out) / VectorE (Schraudolph int8
    bit-trick) ~53/47 Bresenham-interleaved.  Inputs land fp8/bf16 (xkv in
    both [c,m] and [m,c] layouts), chunked DMAs so projections overlap the
    loads.  Attention contributes O(1e-4) of the output, so fp8 noise is
    invisible; the residual is bf16 (0.2% of tolerance).
"""

import contextlib

import numpy as np
import ml_dtypes

import concourse.bass as bass
import concourse.mybir as mybir
from concourse import bacc
from concourse.tile import TileContext
from concourse.bass_utils import run_bass_kernel_spmd

F32 = mybir.dt.float32
BF16 = mybir.dt.bfloat16
FP8 = mybir.dt.float8e4
I8 = mybir.dt.int8
AF = mybir.ActivationFunctionType
DR = mybir.MatmulPerfMode.DoubleRow
ALU = mybir.AluOpType

C = 128           # channels
RC = 16           # reduced (q/k) channels
D = H = W = 24
N = D * H * W     # 13824 tokens
NCORES = 8
NQ = N // NCORES  # 1728 queries per core
CHUNK = 432       # query chunk ([128, CHUNK] f32 fits half a PSUM slot)
NCHUNKS = NQ // CHUNK   # 4
MT = N // 128     # 108 key tiles of 128
PAIRS = MT // 2   # 54 key-tile pairs per chunk
LAGP = 6          # Z/rs matmuls trail exp by this many pairs

KTW = 512         # k-projection column width per matmul (one PSUM bank)
KGROUPS = 3       # k output-row groups per PSUM tile (base partitions 0/32/64)
KTILES = N // (KTW * KGROUPS)   # 9

LOG2E = 1.4426950408889634
EXP8_SCALE = 8.0 * LOG2E / 16.0   # e4m3 bit trick, folding the S'=16*S scale
EXP8_BIAS = 56.0 - 0.3            # 7*8 + Schraudolph offset
ACT_FRAC = 0.531                  # ScalarE share of exp ops (1025/(905+1025))


def _act_pattern(n):
    pat, acc = [], 0.0
    for _ in range(n):
        acc += ACT_FRAC
        if acc >= 1.0:
            acc -= 1.0
            pat.append(True)
        else:
            pat.append(False)
    return pat

_BUILD_CACHE: dict = {}


def build_nc(repeats: int = 1):
    """Build + compile the per-core Bass program (SPMD across 8 cores)."""
    key = repeats
    if key in _BUILD_CACHE:
        return _BUILD_CACHE[key]

    nc = bacc.Bacc("TRN2", target_bir_lowering=False, debug=False,
                   num_devices=NCORES)
    wbf = nc.dram_tensor("wbf", [C, 2 * RC + C], BF16, kind="ExternalInput").ap()
    bias2 = nc.dram_tensor("bias2", [C, 3], F32, kind="ExternalInput").ap()
    xkv_f8 = nc.dram_tensor("xkv_f8", [C, N], FP8, kind="ExternalInput").ap()
    xq_bf = nc.dram_tensor("xq_bf", [C, NQ], BF16, kind="ExternalInput").ap()
    xkvT = nc.dram_tensor("xkvT", [C, N], FP8, kind="ExternalInput").ap()
    y = nc.dram_tensor("y", [C, NQ], BF16, kind="ExternalOutput").ap()

    with TileContext(nc) as tc, contextlib.ExitStack() as ctx:
        cpool = ctx.enter_context(tc.tile_pool(name="consts", bufs=1))
        ppool = ctx.enter_context(tc.tile_pool(name="psum", bufs=1, space="PSUM"))
        spool = ctx.enter_context(tc.tile_pool(name="work", bufs=1))

        # ---- resident inputs (issue order == HWDGE order: critical first) --
        KT1 = KTW * KGROUPS          # one k-tile's worth of xkv columns
        xkv_sb = cpool.tile([C, N], FP8)
        # first k-tile's columns + chunk-0 queries lead; weights right behind
        nc.sync.dma_start(xkv_sb[:, 0:KT1], xkv_f8[:, 0:KT1])
        xq_sb = cpool.tile([C, NQ], BF16)
        nc.sync.dma_start(xq_sb[:, 0:CHUNK], xq_bf[:, 0:CHUNK])
        wbf_sb = cpool.tile([C, 2 * RC + C], BF16)
        nc.sync.dma_start(wbf_sb[:], wbf[:])
        bias_sb = cpool.tile([C, 3], F32)
        nc.sync.dma_start(bias_sb[:], bias2[:])
        nc.sync.dma_start(xkv_sb[:, KT1:2 * KT1], xkv_f8[:, KT1:2 * KT1])
        nc.sync.dma_start(xq_sb[:, CHUNK:NQ], xq_bf[:, CHUNK:NQ])
        XH = (N - 2 * KT1) // 2
        nc.sync.dma_start(xkv_sb[:, 2 * KT1:2 * KT1 + XH],
                          xkv_f8[:, 2 * KT1:2 * KT1 + XH])
        xkvT_sb = cpool.tile([C, N], FP8)

        def xkv_tail2():
            nc.sync.dma_start(xkv_sb[:, 2 * KT1 + XH:N],
                              xkv_f8[:, 2 * KT1 + XH:N])

        wqT = wbf_sb[:, 0:RC]
        wkT = wbf_sb[:, RC:2 * RC]
        wvT = wbf_sb[:, 2 * RC:2 * RC + C]
        bk16 = bias_sb[:, 0:1]
        bq_lo = bias_sb[0:8, 1:2]   # bq[p]/4 on partition p
        bq_hi = bias_sb[0:8, 2:3]   # bq[8+p]/4 on partition p

        ones_db = cpool.tile([C, 32], FP8)
        nc.gpsimd.memset(ones_db[:], 0.0625)   # folds the Z-evac 1/16 scale
        ones_row = cpool.tile([1, C], BF16)  # lhsT for 1->128 broadcast matmul
        nc.gpsimd.memset(ones_row[:], 1.0)
        warm_mv = cpool.tile([1, 512], BF16)
        nc.gpsimd.memset(warm_mv[:], 0.0)

        # PE p-state warmup: keep PE busy early so projection matmuls run at
        # full clock once their inputs land.
        warm_ps = ppool.tile([C, 512], F32, tag="rs", bufs=1, name="warm_ps")
        for _ in range(6):
            nc.tensor.matmul(warm_ps[0:1, :], ones_row[:, 0:1], warm_mv[:],
                             start=True, stop=True)


        def st_slot():
            t = ppool.tile([C, 1024], F32, tag="st", bufs=3, name="stslot")
            return 0, t

        # greedy engine balancing on projected finish time
        eng_t = {"act": 1283.0, "dve": 0.0}  # ACT starts with the table load

        def pick(cost_act, cost_dve):
            if eng_t["act"] + cost_act <= eng_t["dve"] + cost_dve:
                eng_t["act"] += cost_act
                return True
            eng_t["dve"] += cost_dve
            return False

        ex_tiles = {}

        def flush_exp():
            pass

        def pair_exp(gp, stp):
            view = stp.rearrange("p (q x) -> p q x", q=2)[:, :, 0:CHUNK]
            ex = spool.tile([C, 2 * CHUNK], FP8, tag="ex2", bufs=LAGP + 3,
                            name="ex")
            exv = ex.rearrange("p (q x) -> p q x", q=2)
            if pick(905.0, 1025.0):
                nc.scalar.activation(exv, view, AF.Exp, scale=1.0 / 16.0)
            else:
                nc.vector.tensor_scalar(out=exv.bitcast(I8), in0=view,
                                        scalar1=EXP8_SCALE, scalar2=EXP8_BIAS,
                                        op0=ALU.mult, op1=ALU.add)
            ex_tiles[gp] = (ex, 2, 0)

        # ---- projections ---------------------------------------------------
        # k': [128, KTILES*KTW] fp8, partition 32g+r holds 16*k[r] for column
        # group g; evacuations run full-width, then SBUF->SBUF DMAs remap to
        # the DoubleRow layout (o=0 on HWDGE/SP, o=1 on SWDGE/Pool).  q': two
        # 8-row matmuls per chunk-group write the DoubleRow halves side by
        # side in PSUM; 8-lane evacs land straight in q_db layout (no remap).
        # Only k tiles 0-1 and q group 0 run before the attention loop; the
        # rest is interleaved into chunk 0's pair pipeline below.
        k_sb = cpool.tile([C, KTILES * KTW], FP8)
        q_db = cpool.tile([8, 2 * NQ], FP8)
        qdv = q_db.rearrange("p (o g m) -> p o g m", o=2, g=NCHUNKS)
        k_db = cpool.tile([8, 2 * N], FP8)
        kv = k_sb.rearrange("p (t m) -> p t m", t=KTILES)
        kdv = k_db.rearrange("p (o t g m) -> p o t g m", o=2, t=KTILES, g=KGROUPS)

        def q_proj(g):
            flush_exp()
            _, psq = st_slot()
            for o in range(2):
                nc.tensor.matmul(psq[0:8, 512 * o:512 * o + CHUNK],
                                 wqT[:, 8 * o:8 * o + 8],
                                 xq_sb[:, bass.ts(g, CHUNK)],
                                 start=True, stop=True)
            for o, b in ((0, bq_lo), (1, bq_hi)):
                src = psq[0:8, 512 * o:512 * o + CHUNK]
                if pick(545, 575):
                    nc.scalar.activation(qdv[:, o, g, :], src, AF.Identity,
                                         bias=b)
                else:
                    nc.vector.tensor_scalar(out=qdv[:, o, g, :], in0=src,
                                            scalar1=b, scalar2=None,
                                            op0=ALU.add)

        def k_tile(t):
            flush_exp()
            _, psk = st_slot()
            for g in range(KGROUPS):
                lo = (KGROUPS * t + g) * KTW
                nc.tensor.matmul(psk[32 * g:32 * g + RC, 0:KTW],
                                 wkT, xkv_sb[:, lo:lo + KTW],
                                 start=True, stop=True)
            dst = k_sb[:, bass.ts(t, KTW)]
            if pick(612, 658):
                nc.scalar.activation(dst, psk[:, 0:KTW], AF.Identity, bias=bk16)
            else:
                nc.vector.tensor_scalar(out=dst, in0=psk[:, 0:KTW],
                                        scalar1=bk16, scalar2=None, op0=ALU.add)

        def k_remap(t0, t1, pool_frac=False):
            # critical remaps all ride HWDGE (SP); late batches push the o=1
            # half through SWDGE (Pool) to keep HWDGE clear for xkvT loads
            tsl = slice(t0, t1)
            for g in range(KGROUPS):
                nc.sync.dma_start(kdv[:, 0, tsl, g, :],
                                  kv[32 * g:32 * g + 8, tsl, :])
                eng = nc.gpsimd if pool_frac else nc.sync
                eng.dma_start(kdv[:, 1, tsl, g, :],
                              kv[32 * g + 8:32 * g + 16, tsl, :])

        def xkvT_load(qq):
            sl = bass.ts(qq, N // 4)
            nc.sync.dma_start(xkvT_sb[:, sl], xkvT[:, sl])

        k_tile(0)
        q_proj(0)
        k_tile(1)
        k_remap(0, 1, True)
        k_remap(1, 2, True)
        xkvT_load(0)

        q3 = q_db.rearrange("p (o x) -> p o x", o=2)
        k3 = k_db.rearrange("p (o x) -> p o x", o=2)
        # remaining projection work, interleaved at chunk-0 pair slots
        extras = {
            0: [lambda: k_tile(2)],
            1: [xkv_tail2],
            2: [lambda: k_tile(3)],
            4: [lambda: k_remap(2, 4, True)],
            5: [lambda: xkvT_load(1)],
            6: [lambda: k_tile(4)],
            8: [lambda: k_tile(5)],
            10: [lambda: k_tile(6)],
            12: [lambda: k_tile(7)],
            14: [lambda: k_tile(8)],
            15: [lambda: q_proj(1), lambda: k_remap(4, KTILES, True)],
            16: [lambda: xkvT_load(2)],
            17: [lambda: q_proj(2)],
            18: [lambda: xkvT_load(3)],
            19: [lambda: q_proj(3)],
        }

        # ---- attention main loop ------------------------------------------
        # The per-chunk epilogue is deferred into the NEXT chunk's pipeline.
        pend = {}

        def epi_a():
            eng_t["act"] += 545.0
            eng_t["dve"] += 575.0
            # evacuate Z (frees its bank for the next chunk) + reciprocal
            pend["z8"] = z8 = spool.tile([C, CHUNK], FP8, tag="z8", bufs=2,
                                         name="z8")
            nc.scalar.activation(z8[:], pend.pop("z")[:], AF.Copy,
                                 scale=1.0 / 16.0)
            recip = spool.tile([1, CHUNK], F32, tag="recip", bufs=2)
            nc.vector.reciprocal_approx_fast(out=recip[:], in_=pend.pop("rs")[:])
            pend["recip_bf"] = recip_bf = spool.tile([1, CHUNK], BF16,
                                                     tag="recipb", bufs=2,
                                                     name="recip_bf")
            nc.gpsimd.tensor_copy(recip_bf[:], recip[:])

        def epi_m():
            # borrow one arena slot: outu (cols 0:CHUNK) + bcp (512:512+CHUNK)
            eng_t["act"] += 545.0
            flush_exp()
            _, pet = st_slot()
            pend["pe_t"] = pet
            nc.tensor.matmul(pet[:, 0:CHUNK], wvT, pend.pop("z8")[:],
                             start=True, stop=True)
            nc.tensor.matmul(pet[:, 512:512 + CHUNK], ones_row[:],
                             pend.pop("recip_bf")[:], start=True, stop=True)
            pend["outu_s"] = outu_s = spool.tile([C, CHUNK], F32, tag="outu_s",
                                                 bufs=2, name="outu_s")
            nc.scalar.copy(outu_s[:], pet[:, 0:CHUNK])

        def epi_b():
            eng_t["dve"] += 575.0
            sl = pend.pop("sl")
            pet = pend.pop("pe_t")
            t1 = spool.tile([C, CHUNK], BF16, tag="t1", bufs=2)
            nc.vector.tensor_tensor(out=t1[:], in0=pet[:, 512:512 + CHUNK],
                                    in1=pend.pop("outu_s")[:], op=ALU.mult)
            res = spool.tile([C, CHUNK], BF16, tag="res", bufs=2)
            nc.gpsimd.tensor_add(res[:], t1[:], xq_sb[:, sl])
            nc.sync.dma_start(y[:, sl], res[:])

        def epi_final(ch):
            # tail-latency version: two column halves pipelined across
            # engines, one y DMA at the end
            z, rs = pend.pop("z"), pend.pop("rs")
            HW_ = CHUNK // 2
            resf = spool.tile([C, CHUNK], BF16, tag="resf", bufs=1, name="rsf")
            for h in range(2):
                lo = h * HW_
                z8h = spool.tile([C, HW_], FP8, tag="z8f", bufs=2, name="z8f")
                nc.scalar.activation(z8h[:], z[:, lo:lo + HW_], AF.Copy,
                                     scale=1.0 / 16.0)
                rch = spool.tile([1, HW_], F32, tag="recf", bufs=2, name="rcf")
                nc.vector.reciprocal_approx_fast(out=rch[:],
                                                 in_=rs[:, lo:lo + HW_])
                rbh = spool.tile([1, HW_], BF16, tag="rbf", bufs=2, name="rbf")
                nc.gpsimd.tensor_copy(rbh[:], rch[:])
                flush_exp()
                _, pet = st_slot()
                nc.tensor.matmul(pet[:, 0:HW_], wvT, z8h[:],
                                 start=True, stop=True)
                nc.tensor.matmul(pet[:, 512:512 + HW_], ones_row[:], rbh[:],
                                 start=True, stop=True)
                osh = spool.tile([C, HW_], F32, tag="osf", bufs=2, name="osf")
                nc.scalar.copy(osh[:], pet[:, 0:HW_])
                t1h = spool.tile([C, HW_], BF16, tag="t1f", bufs=2, name="t1f")
                nc.vector.tensor_tensor(out=t1h[:], in0=pet[:, 512:512 + HW_],
                                        in1=osh[:], op=ALU.mult)
                cl = bass.ds(ch * CHUNK + lo, HW_)
                nc.vector.tensor_tensor(out=resf[:, lo:lo + HW_], in0=t1h[:],
                                        in1=xq_sb[:, cl], op=ALU.add)
            nc.sync.dma_start(y[:, bass.ts(ch, CHUNK)], resf[:])

        TOT = NCHUNKS * PAIRS
        o3 = ones_db.rearrange("p (b c) -> p b c", b=2)[:, :, 0:1]
        for rep in range(repeats):
            zcur = {}
            for gp in range(TOT + LAGP):
                if rep == 0:
                    for f in extras.get(gp, ()):
                        f()
                s2 = gp - LAGP
                if s2 >= 0:
                    sp = s2 % PAIRS
                    if sp == 0 and "z" in pend:
                        epi_a()
                    if sp == 2 and "z8" in pend:
                        epi_m()
                    if sp == 4 and "pe_t" in pend:
                        epi_b()
                if gp < TOT:
                    ch = gp // PAIRS
                    s = gp % PAIRS
                    sl = bass.ts(ch, CHUNK)
                    i, stp = st_slot()
                    for j in range(2):
                        t = 2 * s + j
                        nc.tensor.matmul(stp[:, 512 * j:512 * j + CHUNK],
                                         k3[:, :, bass.ts(t, 128)],
                                         q3[:, :, sl],
                                         start=True, stop=True, perf_mode=DR)
                    pair_exp(gp, stp)
                if s2 >= 0:
                    ch2 = s2 // PAIRS
                    s = s2 % PAIRS
                    if s == 0:
                        zcur["z"] = ppool.tile([C, CHUNK], F32, tag="z",
                                               name="zpsum")
                        zcur["rs"] = ppool.tile([1, CHUNK], F32, tag="rs",
                                                name="rspsum")
                    ex, qw, qb = ex_tiles.pop(s2)
                    ex3 = ex.rearrange("p (q x) -> p q x",
                                       q=qw)[:, qb:qb + 2, 0:CHUNK]
                    xt3 = xkvT_sb[:, bass.ds(256 * s, 256)].rearrange(
                        "p (b c) -> p b c", b=2)
                    nc.tensor.matmul(zcur["z"][:], xt3, ex3, perf_mode=DR,
                                     start=(s == 0), stop=(s == PAIRS - 1))
                    nc.tensor.matmul(zcur["rs"][:], o3, ex3, perf_mode=DR,
                                     start=(s == 0), stop=(s == PAIRS - 1))
                    if s == PAIRS - 1:
                        pend.update(z=zcur.pop("z"), rs=zcur.pop("rs"),
                                    sl=bass.ts(ch2, CHUNK))
            if rep != repeats - 1:
                epi_a()
                epi_m()
                epi_b()
                tc.strict_bb_all_engine_barrier()
        if "z" in pend:
            pend.pop("sl")
            epi_final(NCHUNKS - 1)

    nc.compile()
    _BUILD_CACHE[key] = nc
    return nc


def _prep_in_maps(x_q, x_kv, Wq, bq, Wk, bk, Wv, bv, gamma):
    bf16 = ml_dtypes.bfloat16
    f8 = ml_dtypes.float8_e4m3
    f32 = np.float32
    x_q = np.asarray(x_q, f32).reshape(C, N)
    x_kv = np.asarray(x_kv, f32).reshape(C, N)
    Wq = np.asarray(Wq, f32)
    bq = np.asarray(bq, f32)
    Wk = np.asarray(Wk, f32)
    bk = np.asarray(bk, f32)
    Wv = np.asarray(Wv, f32)
    bv = np.asarray(bv, f32)
    gamma = float(np.asarray(gamma, f32).reshape(()))

    xkv_f8 = np.ascontiguousarray(x_kv).astype(f8)
    # xkv transposed [m, c] tiled by 128 keys (Z matmul stationary)
    xkvT = np.ascontiguousarray(
        x_kv.reshape(C, MT, 128).transpose(2, 1, 0).reshape(128, MT * C)).astype(f8)
    # bf16 weights blob: Wq^T/4 | 16*Wk^T | gamma*Wv^T
    wbf = np.zeros((C, 2 * RC + C), f32)
    wbf[:, 0:RC] = Wq.T * 0.25
    wbf[:, RC:2 * RC] = Wk.T * 16.0
    wbf[:, 2 * RC:2 * RC + C] = (gamma * Wv).T
    wbf = np.ascontiguousarray(wbf).astype(bf16)
    # biases: col0 = 16*bk tiled at rows 32g+r ; col1/col2 = bq/4 halves
    bias2 = np.zeros((C, 3), f32)
    for g in range(4):
        bias2[32 * g:32 * g + RC, 0] = 16.0 * bk
    bias2[0:8, 1] = 0.25 * bq[0:8]
    bias2[0:8, 2] = 0.25 * bq[8:16]
    resid_bias = (gamma * bv).astype(f32)  # softmax rows sum to 1

    in_maps = []
    for c in range(NCORES):
        xq_slice = np.ascontiguousarray(
            x_q[:, c * NQ:(c + 1) * NQ] + resid_bias[:, None]).astype(bf16)
        in_maps.append({
            "wbf": wbf, "bias2": bias2,
            "xkv_f8": xkv_f8, "xq_bf": xq_slice, "xkvT": xkvT,
        })
    return in_maps


def kernel(x_q, x_kv, Wq, bq, Wk, bk, Wv, bv, gamma):
    nc = build_nc(repeats=1)
    in_maps = _prep_in_maps(x_q, x_kv, Wq, bq, Wk, bk, Wv, bv, gamma)
    res = run_bass_kernel_spmd(nc, in_maps, list(range(NCORES)))
    out = np.concatenate([res.results[c]["y"].astype(np.float32)
                          for c in range(NCORES)], axis=1)
    return out.reshape(1, C, D, H, W).astype(np.float32)
